# revision 5
# baseline (speedup 1.0000x reference)
"""Bass/Trainium2 kernel for nn_JointFocalAttention — 8 NeuronCores, batch-parallel.

Per core (one batch): channel-major projections (host-pre-transposed weights),
depthwise focal convs split across TensorE (per-tap diagonal matmuls in bf16,
clipped 2D APs, f32 PSUM accumulation) and VectorE (fused scalar_tensor_tensor
MACs in f32), gelu + spatial-mean fused on ScalarE, softmax with fused row-sums
(activation accum_out), normalized probs cast to bf16 and DMA-xbar-transposed
for the PV matmuls, final projection on TensorE.
"""
from contextlib import ExitStack

import numpy as np

B, N_L, C, NH = 8, 256, 256, 8
HH, WW = 64, 64
N_C = HH * WW
L = 3
KS = [3, 5, 7]
HD = C // NH
SCALE = HD ** -0.5
N_CORES = 8
PE_CHUNKS = 6   # of 8 spatial row-chunks on TensorE (rest on VectorE)
N_CHUNK = 8     # rows per chunk
NCH = N_C // 8  # 512

_STATE = {}


def _taps(k):
    p = k // 2
    return [(dy, dx) for dy in range(-p, p + 1) for dx in range(-p, p + 1)]


def _build_nc():
    import concourse.tile as tile
    import concourse.mybir as mybir
    from concourse import bacc

    F32 = mybir.dt.float32
    BF16 = mybir.dt.bfloat16
    ALU = mybir.AluOpType
    AF = mybir.ActivationFunctionType
    AX = mybir.AxisListType

    nc = bacc.Bacc("TRN2", target_bir_lowering=False, debug=False,
                   num_devices=N_CORES)

    def din(name, shape, dt=F32):
        return nc.dram_tensor(name, shape, dt, kind="ExternalInput").ap()

    latT = din("latT", [C, N_L])
    ctxT = din("ctxT", [C, N_C])
    qwT = din("qwT", [C, C])
    kwT = din("kwT", [C, C])
    fwT = din("fwT", [C, C + L + 1])
    hwT = din("hwT", [C, C])
    pwT = din("pwT", [C, C])
    fb = din("fb", [C + L + 1, 1])
    hb = din("hb", [C, 1])
    dgs = [din(f"dg{i}", [128, len(_taps(KS[i])) * 2, 128], BF16)
           for i in range(L)]
    wcs = [din(f"wc{i}", [128, 2, len(_taps(KS[i]))]) for i in range(L)]

    attn_out = nc.dram_tensor("attn", [NH, N_L, N_C], F32,
                              kind="ExternalOutput").ap()
    out_out = nc.dram_tensor("out", [N_L, C], F32, kind="ExternalOutput").ap()
    kT_dram = nc.dram_tensor("kT_spill", [C, N_C], F32).ap()
    vmod_dram = nc.dram_tensor("vmod_spill", [C, N_C], F32).ap()
    gates_dram = nc.dram_tensor("gates_spill", [4, N_C], F32).ap()

    with tile.TileContext(nc) as tc, ExitStack() as ctx:
        big = ctx.enter_context(tc.tile_pool(name="big", bufs=6))
        med = ctx.enter_context(tc.tile_pool(name="med", bufs=1))
        md2 = ctx.enter_context(tc.tile_pool(name="md2", bufs=2))
        sml = ctx.enter_context(tc.tile_pool(name="sml", bufs=3))
        dgp = ctx.enter_context(tc.tile_pool(name="dgp", bufs=1))
        v16p = ctx.enter_context(tc.tile_pool(name="v16p", bufs=1))
        e16p = ctx.enter_context(tc.tile_pool(name="e16p", bufs=1))
        etp = ctx.enter_context(tc.tile_pool(name="etp", bufs=1))
        vpvp = ctx.enter_context(tc.tile_pool(name="vpvp", bufs=1))
        dvp = ctx.enter_context(tc.tile_pool(name="dvp", bufs=1))
        psp = ctx.enter_context(tc.tile_pool(name="psp", bufs=1, space="PSUM"))

        def pbank(i, shape=None):
            return psp.tile(shape or [128, NCH], F32, tag=f"bank{i}",
                            name=f"pb{i}")

        # ---------------- Phase A ----------------
        ctx_sb = [big.tile([128, N_C], F32, tag="b4k", name=f"ctx{i}")
                  for i in range(2)]
        nc.sync.dma_start(ctx_sb[0][:], ctxT[0:128, :])
        nc.sync.dma_start(ctx_sb[1][:], ctxT[128:256, :])
        latT_sb = med.tile([128, 2, N_L], F32, tag="latT")
        nc.sync.dma_start(latT_sb[:], latT[:].rearrange("(t p) q -> p t q", t=2))
        qwT_sb = med.tile([128, 2, C], F32, tag="wq")
        nc.sync.dma_start(qwT_sb[:], qwT[:].rearrange("(t p) j -> p t j", t=2))
        kwT_sb = med.tile([128, 2, C], F32, tag="wk")
        nc.sync.dma_start(kwT_sb[:], kwT[:].rearrange("(t p) j -> p t j", t=2))
        fwT_sb = med.tile([128, 2, C + L + 1], F32, tag="wf")
        nc.sync.dma_start(fwT_sb[:], fwT[:].rearrange("(t p) j -> p t j", t=2))
        fb_sb = sml.tile([128, 3], F32, tag="fb")
        nc.vector.memset(fb_sb[:], 0.0)
        nc.sync.dma_start(fb_sb[0:128, 0:1], fb[0:128])
        nc.sync.dma_start(fb_sb[0:128, 1:2], fb[128:256])
        nc.sync.dma_start(fb_sb[0:4, 2:3], fb[256:260])

        qT_sb = med.tile([128, 2, N_L], F32, tag="qT")
        for mt in range(2):
            pt = pbank(6, [128, N_L])
            for kt in range(2):
                nc.tensor.matmul(pt[:], qwT_sb[:, kt, 128 * mt:128 * (mt + 1)],
                                 latT_sb[:, kt, :],
                                 start=(kt == 0), stop=(kt == 1))
            nc.scalar.copy(qT_sb[:, mt, :], pt[:])

        for mt in range(2):
            kt_sb = big.tile([128, N_C], F32, tag="b4k")
            for ch in range(8):
                pt = pbank(4 + ch % 2)
                for kt in range(2):
                    nc.tensor.matmul(
                        pt[:], kwT_sb[:, kt, 128 * mt:128 * (mt + 1)],
                        ctx_sb[kt][:, NCH * ch:NCH * (ch + 1)],
                        start=(kt == 0), stop=(kt == 1))
                nc.scalar.copy(kt_sb[:, NCH * ch:NCH * (ch + 1)], pt[:])
            nc.sync.dma_start(kT_dram[128 * mt:128 * (mt + 1), :], kt_sb[:])

        v_prev = [big.tile([128, N_C], F32, tag="b4k", name=f"vp{i}")
                  for i in range(2)]
        for mt in range(3):
            mrows = 128 if mt < 2 else 4
            for ch in range(8):
                pt = pbank(4 + ch % 2)
                for kt in range(2):
                    nc.tensor.matmul(
                        pt[0:mrows, :],
                        fwT_sb[:, kt, 128 * mt:128 * mt + mrows],
                        ctx_sb[kt][:, NCH * ch:NCH * (ch + 1)],
                        start=(kt == 0), stop=(kt == 1))
                if mt < 2:
                    nc.scalar.activation(v_prev[mt][:, NCH * ch:NCH * (ch + 1)],
                                         pt[0:mrows, :], AF.Identity,
                                         bias=fb_sb[0:mrows, mt:mt + 1])
                else:
                    gtmp = sml.tile([4, NCH], F32, tag="gtmp")
                    nc.scalar.activation(gtmp[:], pt[0:4, :], AF.Identity,
                                         bias=fb_sb[0:4, 2:3])
                    nc.sync.dma_start(
                        gates_dram[:, NCH * ch:NCH * (ch + 1)], gtmp[:])

        # ---------------- Phase B: focal conv ----------------
        v_all = [big.tile([128, N_C], F32, tag="b4k", name=f"va{i}")
                  for i in range(2)]
        vg_sums = sml.tile([128, 2, PE_CHUNKS + 1], F32, tag="vgs")
        for lvl in range(L):
            taps = _taps(KS[lvl])
            nt = len(taps)
            dgt = dgp.tile([128, nt * 2, 128], BF16, tag="diag")
            nc.sync.dma_start(dgt[:], dgs[lvl][:])
            wct = sml.tile([128, 2, 64], F32, tag="wc")
            nc.sync.dma_start(wct[:, :, 0:nt], wcs[lvl][:])
            v_cur = [big.tile([128, N_C], F32, tag="b4k", name=f"vc{i}")
                      for i in range(2)]
            last_lvl = lvl == L - 1
            for hf in range(2):
                v16 = v16p.tile([128, N_C], BF16, tag="v16")
                nc.gpsimd.tensor_copy(v16[:], v_prev[hf][:])
                src16 = v16[:].rearrange("c (h w) -> c h w", h=HH)
                src = v_prev[hf][:].rearrange("c (h w) -> c h w", h=HH)
                dst3 = v_cur[hf][:].rearrange("c (h w) -> c h w", h=HH)
                for cc in range(PE_CHUNKS):
                    r0 = cc * N_CHUNK
                    pt = pbank(cc % 4, [128, N_CHUNK, WW])
                    valid = [(ti, dy, dx,
                              max(r0, -dy), min(r0 + N_CHUNK, HH - dy),
                              max(0, -dx), min(WW, WW - dx))
                             for ti, (dy, dx) in enumerate(taps)]
                    valid = [v for v in valid if v[3] < v[4]]
                    for i, (ti, dy, dx, y0, y1, x0, x1) in enumerate(valid):
                        nc.tensor.matmul(
                            pt[:, y0 - r0:y1 - r0, x0:x1],
                            dgt[:, 2 * ti + hf, :],
                            src16[:, y0 + dy:y1 + dy, x0 + dx:x1 + dx],
                            start=(i == 0), stop=(i == len(valid) - 1))
                    acc = vg_sums[:, hf, cc:cc + 1] if last_lvl else None
                    nc.scalar.activation(dst3[:, r0:r0 + N_CHUNK, :], pt[:],
                                         AF.Gelu, accum_out=acc)
                R0 = PE_CHUNKS * N_CHUNK
                if R0 < HH:
                    cv = dvp.tile([128, HH - R0, WW], F32, tag="dvacc")
                    nc.vector.memset(cv[:], 0.0)
                    for ti, (dy, dx) in enumerate(taps):
                        y0, y1 = max(R0, -dy), min(HH, HH - dy)
                        x0, x1 = max(0, -dx), min(WW, WW - dx)
                        if y0 >= y1:
                            continue
                        s = src[:, y0 + dy:y1 + dy, x0 + dx:x1 + dx]
                        d = cv[:, y0 - R0:y1 - R0, x0:x1]
                        nc.vector.scalar_tensor_tensor(
                            d, s, wct[:, hf, ti:ti + 1], d, ALU.mult, ALU.add)
                    acc2 = (vg_sums[:, hf, PE_CHUNKS:PE_CHUNKS + 1]
                            if last_lvl else None)
                    nc.scalar.activation(dst3[:, R0:HH, :], cv[:], AF.Gelu,
                                         accum_out=acc2)
                for ch in range(8):
                    sl = slice(NCH * ch, NCH * (ch + 1))
                    gb = sml.tile([128, NCH], F32, tag="gbc")
                    nc.sync.dma_start(
                        gb[:],
                        gates_dram[lvl:lvl + 1, sl].partition_broadcast(128))
                    if lvl == 0:
                        nc.vector.tensor_mul(v_all[hf][:, sl],
                                             v_cur[hf][:, sl], gb[:])
                    else:
                        tmp = md2.tile([128, NCH], F32, tag="gtm2")
                        nc.vector.tensor_mul(tmp[:], v_cur[hf][:, sl], gb[:])
                        nc.vector.tensor_add(v_all[hf][:, sl],
                                             v_all[hf][:, sl], tmp[:])
            v_prev = v_cur

        for hf in range(2):
            ssum = sml.tile([128, 1], F32, tag="ssum")
            nc.vector.tensor_reduce(ssum[:], vg_sums[:, hf, :], AX.X, ALU.add)
            vglob = sml.tile([128, 1], F32, tag="vglob")
            nc.scalar.activation(vglob[:], ssum[:], AF.Gelu, scale=1.0 / N_C)
            for ch in range(8):
                sl = slice(NCH * ch, NCH * (ch + 1))
                gb = sml.tile([128, NCH], F32, tag="gbc")
                nc.sync.dma_start(
                    gb[:], gates_dram[3:4, sl].partition_broadcast(128))
                nc.vector.scalar_tensor_tensor(
                    v_all[hf][:, sl], gb[:], vglob[:], v_all[hf][:, sl],
                    ALU.mult, ALU.add)

        # ---------------- Phase C ----------------
        hwT_sb = med.tile([128, 2, C], F32, tag="wh")
        nc.sync.dma_start(hwT_sb[:], hwT[:].rearrange("(t p) j -> p t j", t=2))
        hb_sb = sml.tile([128, 2], F32, tag="hb")
        nc.sync.dma_start(hb_sb[:, 0:1], hb[0:128])
        nc.sync.dma_start(hb_sb[:, 1:2], hb[128:256])
        for mt in range(2):
            vm_sb = big.tile([128, N_C], F32, tag="b4k")
            for ch in range(8):
                pt = pbank(4 + ch % 2)
                for kt in range(2):
                    nc.tensor.matmul(
                        pt[:], hwT_sb[:, kt, 128 * mt:128 * (mt + 1)],
                        v_all[kt][:, NCH * ch:NCH * (ch + 1)],
                        start=(kt == 0), stop=(kt == 1))
                nc.scalar.activation(vm_sb[:, NCH * ch:NCH * (ch + 1)], pt[:],
                                     AF.Identity, bias=hb_sb[:, mt:mt + 1])
            nc.sync.dma_start(vmod_dram[128 * mt:128 * (mt + 1), :], vm_sb[:])
        v_pv = vpvp.tile([128, 32, 256], BF16, tag="vpv")
        for t in range(32):
            nc.gpsimd.dma_start(
                v_pv[:, t, :],
                vmod_dram[8 * t:8 * (t + 1), :].rearrange(
                    "c (r j) -> (c r) j", r=16))

        # ---------------- Phase D: attention ----------------
        pwT_sb = med.tile([128, 2, C], F32, tag="wp")
        nc.sync.dma_start(pwT_sb[:], pwT[:].rearrange("(t p) j -> p t j", t=2))
        outT_sb = med.tile([128, 2, N_L], F32, tag="outT")

        for qt in range(2):
            for hg in range(2):
                group = []
                for hh in range(4):
                    e_sb = big.tile([128, N_C], F32, tag="b4k")
                    sums = sml.tile([128, 8], F32, tag=f"sums{hh}")
                    group.append((hg * 4 + hh, hh, e_sb, sums))
                for ch in range(8):
                    ktch = md2.tile([128, NCH], F32, tag="ktch")
                    nc.sync.dma_start(
                        ktch[:],
                        kT_dram[128 * hg:128 * (hg + 1),
                                NCH * ch:NCH * (ch + 1)])
                    pts = []
                    for (h, hh, e_sb, sums) in group:
                        pt = pbank(hh)
                        nc.tensor.matmul(
                            pt[:],
                            qT_sb[32 * hh:32 * (hh + 1), hg,
                                  128 * qt:128 * (qt + 1)],
                            ktch[32 * hh:32 * (hh + 1), :],
                            start=True, stop=True,
                            tile_position=(32 * hh, 0))
                        pts.append(pt)
                    for (h, hh, e_sb, sums), pt in zip(group, pts):
                        nc.scalar.activation(
                            e_sb[:, NCH * ch:NCH * (ch + 1)], pt[:], AF.Exp,
                            scale=SCALE, accum_out=sums[:, ch:ch + 1])
                for (h, hh, e_sb, sums) in group:
                    rsum = sml.tile([128, 1], F32, tag=f"rsum{hh}")
                    nc.vector.tensor_reduce(rsum[:], sums[:], AX.X, ALU.add)
                    rcp = sml.tile([128, 1], F32, tag=f"rcp{hh}")
                    nc.vector.reciprocal(rcp[:], rsum[:])
                    at_sb = big.tile([128, N_C], F32, tag="b4k")
                    nc.vector.tensor_scalar_mul(at_sb[:], e_sb[:], rcp[:])
                    nc.sync.dma_start(
                        attn_out[h, 128 * qt:128 * (qt + 1), :], at_sb[:])
                    e16 = e16p.tile([128, N_C], BF16, tag="e16")
                    nc.gpsimd.tensor_copy(e16[:], at_sb[:])
                    et = etp.tile([128, 32, 128], BF16, tag="et")
                    for t in range(32):
                        nc.sync.dma_start_transpose(
                            et[:, t, :], e16[:, 128 * t:128 * (t + 1)])
                    pv = pbank(7, [32, 128])
                    for t in range(32):
                        nc.tensor.matmul(
                            pv[:], v_pv[:, t, 32 * h:32 * (h + 1)],
                            et[:, t, :], start=(t == 0), stop=(t == 31))
                    nc.vector.tensor_copy(
                        outT_sb[32 * hh:32 * (hh + 1), hg,
                                128 * qt:128 * (qt + 1)], pv[:])

        for qt in range(2):
            pt = pbank(6, [128, C])
            for kt in range(2):
                nc.tensor.matmul(pt[:], outT_sb[:, kt, 128 * qt:128 * (qt + 1)],
                                 pwT_sb[:, kt, :],
                                 start=(kt == 0), stop=(kt == 1))
            o_sb = md2.tile([128, C], F32, tag="osb")
            nc.scalar.copy(o_sb[:], pt[:])
            nc.sync.dma_start(out_out[128 * qt:128 * (qt + 1), :], o_sb[:])

    nc.compile()
    return nc


def _host_prep(inputs):
    import ml_dtypes
    lat = np.asarray(inputs["latents"], np.float32)
    ctxv = np.asarray(inputs["context"], np.float32)
    shared = {
        "qwT": np.ascontiguousarray(np.asarray(inputs["q_w"], np.float32).T),
        "kwT": np.ascontiguousarray(np.asarray(inputs["k_w"], np.float32).T),
        "fwT": np.ascontiguousarray(np.asarray(inputs["f_w"], np.float32).T),
        "hwT": np.ascontiguousarray(np.asarray(inputs["h_w"], np.float32).T),
        "pwT": np.ascontiguousarray(np.asarray(inputs["proj_w"], np.float32).T),
        "fb": np.asarray(inputs["f_b"], np.float32).reshape(C + L + 1, 1),
        "hb": np.asarray(inputs["h_b"], np.float32).reshape(C, 1),
    }
    for i, k in enumerate(KS):
        nt = k * k
        w = np.asarray(inputs[f"fk{i}"], np.float32).reshape(C, nt)
        dg = np.zeros((128, nt * 2, 128), np.float32)
        wc = np.zeros((128, 2, nt), np.float32)
        for hf in range(2):
            for t in range(nt):
                np.fill_diagonal(dg[:, 2 * t + hf, :],
                                 w[128 * hf:128 * (hf + 1), t])
            wc[:, hf, :] = w[128 * hf:128 * (hf + 1), :]
        shared[f"dg{i}"] = dg.astype(ml_dtypes.bfloat16)
        shared[f"wc{i}"] = wc
    in_maps = []
    for b in range(B):
        m = dict(shared)
        m["latT"] = np.ascontiguousarray(lat[b].T)
        m["ctxT"] = np.ascontiguousarray(ctxv[b].T)
        in_maps.append(m)
    return in_maps


def _get_built():
    if "nc" not in _STATE:
        _STATE["nc"] = _build_nc()
    return _STATE["nc"]


def run_on_device(in_maps):
    from concourse.bass_utils import run_bass_kernel_spmd
    nc = _get_built()
    return run_bass_kernel_spmd(nc, in_maps, core_ids=list(range(N_CORES)))


def kernel(**inputs):
    in_maps = _host_prep(inputs)
    res = run_on_device(in_maps)
    attn = np.stack([res.results[b]["attn"] for b in range(B)], axis=0)
    out = np.stack([res.results[b]["out"] for b in range(B)], axis=0)
    pb = np.asarray(inputs["proj_b"], np.float32)
    out = out + pb[None, None, :]
    return out, attn
